# revision 96
# baseline (speedup 1.0000x reference)
"""Trainium2 Bass kernel v3 for nn_BattleModel (segment_reduce).

Per-core architecture (SPMD-identical program on 8 cores, 2048 segments
per core dealt round-robin by global size rank so the cross-core
envelope is tight):

- Unit MLP h=relu(x@W1+b1) on TensorE as K=20 f16 matmuls; each rhs
  column carries TWO units (lanes 0:10 -> out rows 0:64, lanes 10:20 ->
  rows 64:128).  Four 20-lane bands at partition pitch 32 pack the rhs.
- PSUM is organized as 2-supertile windows with SEPARATE A-bank and
  B-bank tiles ([128, 1024] f32 x2 pools, double buffered) so each
  tile has a single reader engine and the sync coalescer cannot chain
  ACT/DVE against each other on tile recycling.  B-bank partner
  columns are packed ONE WINDOW EARLY (host-side layout shift), so the
  window-w STT consumes the ring evicted in window w-1 and each psum
  tile frees within its own window.
- Transit per window: ACT relu-evicts the whole B window into an f16
  ring (one merged op); DVE scalar_tensor_tensor writes
  h16 = relu(A) + ring (pair-add fused into the transit).  A planner
  can flip trailing groups to "split" mode (ACT evicts both banks, the
  pair-add moves into the tree) to rebalance ACT/DVE/Pool.
- Equal-m groups are sub-split (~2.5k alphas, window-aligned) and
  packed into side-major REGION tiles (~3k alphas) so eviction ops
  merge across group borders; trees unlock per region.
- Segment pooling: in-place ragged halving trees in f16 (DVE 2x /
  GpSimd); each group's whole tree runs on ONE engine (picked
  greedily) to avoid cross-engine head-of-line blocking; reduced to
  FOLD=4 pooled cols/segment.
- Combine MLP per chunk (512/512/512/256/256 segments, staged with a
  delay so ACT never head-of-line blocks): Wc1 rows duplicated so the
  A+B partition fold rides the PSUM accumulation; relu+bias and
  sigmoid on ACT, host inverse permutation.
- A dependency-free warm matmul (memset inputs) starts the PE p-state
  ramp at t~0 so the first real matmuls run at full clock.
"""

from contextlib import ExitStack

import numpy as np

import concourse.bacc as bacc
import concourse.bass as bass
import concourse.tile as tile
from concourse import mybir
from concourse.bass_utils import run_bass_kernel_spmd

N_UNITS = 524288
BATCH = 16384
N_CORES = 8
SEG_PER_CORE = BATCH // N_CORES  # 2048
NFEAT = 9
FOLD = 4
TREE_CHUNK = 2048
CH_BNDS = (0, 512, 1024, 1536, 1792, 2048)
NCHUNK = len(CH_BNDS) - 1


def chunk_index(j):
    c = 0
    while CH_BNDS[c + 1] <= j:
        c += 1
    return c

# planner effective rates (ns per 128-row column), incl. amortized init
# (calibrated against TimelineSim engine busies of this kernel)
ACT_R = 1.03
STT_R = 1.29
TD_R = 0.55    # DVE f16 tree add (2x)
TP_R = 2.05    # Pool f16 tree add
ACT_FIXED = 5200.0  # sigmoid + combine relu + warm acts
DVE_FIXED = 200.0   # (combine relu moved to ACT)

# emission-time cost accounting (greedy DVE/Pool balance)
C_DVE_TT = (0.52, 70.0)     # per-col, per-op
C_DVE_CP = (0.26, 70.0)
C_POOL_TT = (1.98, 130.0)
C_POOL_CP = (1.39, 130.0)
SPLIT_PREFIX_A = 0


def host_prep(left_feats, right_feats, left_seg, right_seg):
    left_feats = np.asarray(left_feats, dtype=np.float32)
    right_feats = np.asarray(right_feats, dtype=np.float32)
    left_seg = np.asarray(left_seg)
    right_seg = np.asarray(right_seg)

    cntL = np.bincount(left_seg, minlength=BATCH).astype(np.int64)
    cntR = np.bincount(right_seg, minlength=BATCH).astype(np.int64)
    K = np.maximum(np.maximum((cntL + 1) // 2, (cntR + 1) // 2), 1)
    m_all = np.maximum((K + 1) // 2, FOLD)

    order_g = np.argsort(-K, kind="stable")
    seg_ids = order_g.reshape(SEG_PER_CORE, N_CORES)  # [rank, core]
    m_env = m_all[seg_ids[:, 0]].astype(np.int64)  # non-increasing

    # groups of equal envelope m; merge tiny groups into the previous
    # (bigger-m) one when the padding cost is small
    groups_raw = []
    j = 0
    while j < SEG_PER_CORE:
        j2 = j
        while j2 < SEG_PER_CORE and m_env[j2] == m_env[j]:
            j2 += 1
        groups_raw.append([int(m_env[j]), j, j2 - j])
        j = j2
    merged = [groups_raw[0]]
    for g in groups_raw[1:]:
        m_prev = merged[-1][0]
        pad_cost = g[2] * (m_prev - g[0])
        if g[2] * g[0] < 192 and pad_cost <= 384:
            merged[-1][2] += g[2]
            m_env[g[1] : g[1] + g[2]] = m_prev
        else:
            merged.append(g)
    base_groups = []
    a = 0
    for m, j0, n in merged:
        base_groups.append((m, j0, n, a))
        a += m * n
    A_tot = a
    A2 = -(-A_tot // 512) * 512
    n_st = A2 // 512
    n_st_phys = n_st + 2  # B banks shifted 2 supertiles (1 window) early

    folds = (4,) * NCHUNK

    # sub-split large groups (~SUBG alphas) so trees unlock early;
    # cuts are aligned to the 1024-alpha window grid so STT ops do not
    # fragment at sub-group borders
    SUBG = 2560
    groups = []
    for (m, j0, n, a0) in base_groups:
        pieces = max(1, -(-(n * m) // SUBG))
        cuts = [j0]
        for pi in range(1, pieces):
            a_cut = a0 + (n * m * pi) // pieces
            a_cut = -(-a_cut // 1024) * 1024  # round up to window grid
            j_cut = j0 + min(n, max(0, -(-(a_cut - a0) // m)))
            if j_cut > cuts[-1] and j_cut < j0 + n:
                cuts.append(j_cut)
        cuts.append(j0 + n)
        for ci in range(len(cuts) - 1):
            j, j2 = cuts[ci], cuts[ci + 1]
            groups.append((m, j, j2 - j, a0 + (j - j0) * m))

    modes = _plan_modes(groups, A2, folds)

    # 4 bands of whole (physical) supertiles
    q, rem = divmod(n_st_phys, 4)
    st_per_band = [q + (1 if b < rem else 0) for b in range(4)]
    st_edges = np.concatenate([[0], np.cumsum(st_per_band)])
    W_b = [st_per_band[b] * 1024 for b in range(4)]
    W0 = max(W_b)

    cores = []
    for d in range(N_CORES):
        ids = seg_ids[:, d]
        core = dict(ids=ids)
        for side, feats, seg, cnt in (
            ("L", left_feats, left_seg, cntL),
            ("R", right_feats, right_seg, cntR),
        ):
            start = np.zeros(BATCH, dtype=np.int64)
            start[1:] = np.cumsum(cnt)[:-1]
            cnt_sorted = cnt[ids]
            src_start_sorted = start[ids]
            tot = int(cnt_sorted.sum())
            i_in_seg = np.arange(tot) - np.repeat(
                np.cumsum(cnt_sorted) - cnt_sorted, cnt_sorted
            )
            src_row = np.repeat(src_start_sorted, cnt_sorted) + i_in_seg
            pair = i_in_seg // 2
            half = i_in_seg % 2
            me = np.repeat(m_env, cnt_sorted)
            ab = np.repeat(
                np.concatenate([[0], np.cumsum(m_env)[:-1]]), cnt_sorted
            )
            in_A = pair < me
            alpha = ab + np.where(in_A, pair, pair - me)
            # A-cols in phys supertile alpha//512+2; B-cols (partners) one
            # window EARLIER at phys supertile alpha//512, +512 bank offset
            P = np.where(
                in_A,
                1024 * (alpha // 512 + 2) + (alpha % 512),
                1024 * (alpha // 512) + (alpha % 512) + 512,
            )
            st = P // 1024
            band = np.searchsorted(st_edges, st, side="right") - 1
            col = P - st_edges[band] * 1024
            row0 = 20 * band + 10 * half
            xh = np.zeros((80, W0), dtype=np.float16)
            f32feats = feats[src_row]
            for f in range(NFEAT):
                xh[row0 + f, col] = f32feats[:, f].astype(np.float16)
            xh[row0 + NFEAT, col] = 1.0
            core["xh" + side] = xh
        cores.append(core)

    aux = (st_edges, W_b, W0, modes, folds)
    return dict(A2=A2, n_st=n_st, aux=aux, groups=groups, cores=cores,
                upc=A2, m_pad=n_st, poolw=aux)


def _plan_modes(groups, A2, folds):
    """Pick fused(0)/split(1) per group — split is a suffix (small-m
    groups) so transit windows stay mode-pure — balancing ACT/DVE/Pool."""
    G = len(groups)
    M = np.array([2.0 * n * m for (m, j0, n, a0) in groups])

    def f_of(g):
        return min(folds)

    tree_base = np.array(
        [2.0 * g[2] * max(g[0] - f_of(g), 0) for g in groups]
    )
    best = (None, float("inf"))
    pe = (4 * A2 + 2 * sum(folds) * 512 + 4 * 512) * 0.4167 + 8000.0
    pref = sum(1 for (m, j0, n, a0) in groups if a0 < SPLIT_PREFIX_A)
    for k in range(G + 1 - pref):  # split the last k groups
        split = np.zeros(G, dtype=bool)
        split[:pref] = True
        if k:
            split[G - k :] = True
        act = ACT_R * float(np.where(split, 2 * M, M).sum()) + ACT_FIXED
        stt = STT_R * float(M[~split].sum())
        tree = float(tree_base.sum() + M[split].sum())
        x = (TP_R * tree - stt - DVE_FIXED) / (TD_R + TP_R)
        x = min(max(x, 0.0), tree)
        dve = stt + DVE_FIXED + TD_R * x
        pool = TP_R * (tree - x)
        T = max(act, dve, pool, pe)
        if T < best[1]:
            best = (tuple(bool(s) for s in split), T)
    return best[0]


def make_weight_arrays(W1, b1, Wc1, bc1, Wc2, bc2):
    W1p = np.concatenate(
        [np.asarray(W1, np.float32), np.asarray(b1, np.float32)[None, :]],
        axis=0,
    )  # [10, 64]
    Wc1 = np.asarray(Wc1, np.float32)
    wts = np.zeros((128, 512), dtype=np.float16)
    for b in range(4):
        for k in range(10):
            wts[32 * b + k, 0:64] = W1p[k].astype(np.float16)
            wts[32 * b + 10 + k, 64:128] = W1p[k].astype(np.float16)
    wts[0:64, 128:160] = Wc1[0:64].astype(np.float16)
    wts[64:128, 128:160] = Wc1[0:64].astype(np.float16)
    wts[0:64, 256:288] = Wc1[64:128].astype(np.float16)
    wts[64:128, 256:288] = Wc1[64:128].astype(np.float16)
    wts[0:32, 384] = np.asarray(Wc2, np.float32)[:, 0].astype(np.float16)
    bias = np.zeros((128, 2), dtype=np.float32)
    bias[0:32, 0] = np.asarray(bc1, np.float32)
    bias[0, 1] = np.asarray(bc2, np.float32)[0]
    return dict(wts=wts, bias=bias)


# ------------------------------------------------------------- bass program

def build_nc(A2, n_st, aux, groups):
    st_edges, W_b, W0, modes, folds = aux
    f16, f32 = mybir.dt.float16, mybir.dt.float32
    nc = bacc.Bacc()
    relu = mybir.ActivationFunctionType.Relu
    sigmoid = mybir.ActivationFunctionType.Sigmoid
    add = mybir.AluOpType.add
    mx = mybir.AluOpType.max

    A_tot = groups[-1][3] + groups[-1][0] * groups[-1][2]

    xh_dram = {
        s: nc.declare_dram_parameter("xh" + s, [80, W0], f16, isOutput=False)
        for s in ("L", "R")
    }
    wts_dram = nc.declare_dram_parameter("wts", [128, 512], f16,
                                         isOutput=False)
    bias_dram = nc.declare_dram_parameter("bias", [128, 2], f32,
                                          isOutput=False)
    out_dram = nc.declare_dram_parameter("out", [1, SEG_PER_CORE], f16,
                                         isOutput=True)

    eng_ns = {"dve": DVE_FIXED, "gps": 0.0, "act": 0.0}

    def band_of(k):
        return int(np.searchsorted(st_edges, k, side="right") - 1)

    def group_pieces(a_lo, a_hi):
        out = []
        pos = a_lo
        for i, (m, j0, n, a0) in enumerate(groups):
            g_lo, g_hi = a0, a0 + n * m
            if g_hi <= pos or g_lo >= a_hi:
                continue
            lo, hi = max(pos, g_lo), min(a_hi, g_hi)
            if lo > pos:
                out.append((None, pos, lo))
            out.append((i, lo, hi))
            pos = hi
        if pos < a_hi:
            out.append((None, pos, a_hi))
        return out

    with tile.TileContext(nc) as tc, ExitStack() as ctx:
        consts = ctx.enter_context(tc.tile_pool(name="consts", bufs=1))
        big = ctx.enter_context(tc.tile_pool(name="big", bufs=1))
        ring_pool = ctx.enter_context(tc.tile_pool(name="ring", bufs=12))

        # warm both activation tables from a constant so the table loads
        # run before any data dependencies
        actwarm = consts.tile([1, 3], f32)
        nc.vector.memset(actwarm[:, 2:3], 0)
        nc.scalar.activation(actwarm[:, 0:1], actwarm[:, 2:3], relu)
        nc.scalar.activation(actwarm[:, 1:2], actwarm[:, 2:3], sigmoid)
        wt = consts.tile([128, 512], f16)
        # unit-MLP lhsT columns first (tiny), combine weights later
        nc.scalar.dma_start(wt[:, 0:128], wts_dram[:, 0:128])
        xt = {}
        for s in ("L", "R"):
            xt[s] = big.tile([128, W0], f16, name="xt" + s)
        # band 0 arrives in fine-grained chunks so the first windows start
        # as early as possible
        cuts0 = [0, 2048, 4096, 8192]
        cuts0 = [c for c in cuts0 if c < W_b[0]] + [W_b[0]]
        for i in range(len(cuts0) - 1):
            for s in ("L", "R"):
                nc.sync.dma_start(
                    xt[s][0:20, cuts0[i] : cuts0[i + 1]],
                    xh_dram[s][0:20, cuts0[i] : cuts0[i + 1]],
                )
        nc.scalar.dma_start(wt[:, 128:512], wts_dram[:, 128:512])
        bt = consts.tile([128, 2], f32)
        nc.scalar.dma_start(bt[:], bias_dram[:])
        for b in range(1, 4):
            for s in ("L", "R"):
                if W_b[b] == 0:
                    continue
                nc.sync.dma_start(
                    xt[s][32 * b : 32 * b + 20, 0 : W_b[b]],
                    xh_dram[s][20 * b : 20 * b + 20, 0 : W_b[b]],
                )

        # regions: consecutive same-mode groups packed to ~REGION_A alphas,
        # sharing one side-major h16 tile so evictions merge across groups
        REGION_A = 3072
        regions = []  # (mode, a_start, a_end, [group indices])
        for i, (m, j0, n, a0) in enumerate(groups):
            nm = n * m
            if (regions and regions[-1][0] == modes[i]
                    and regions[-1][2] == a0
                    and regions[-1][2] - regions[-1][1] < REGION_A):
                regions[-1] = (modes[i], regions[-1][1], a0 + nm,
                               regions[-1][3] + [i])
            else:
                regions.append((modes[i], a0, a0 + nm, [i]))
        region_of = {}
        for ri, (md, ra0, ra1, gis) in enumerate(regions):
            for gi in gis:
                region_of[gi] = ri
        rtiles = []
        for ri, (md, ra0, ra1, gis) in enumerate(regions):
            ra = ra1 - ra0
            w = (4 if md else 2) * ra
            rtiles.append(big.tile([128, w], f16, name=f"h16r{ri}"))
        pooled4c = [
            big.tile([128, 2 * folds[c] * (CH_BNDS[c + 1] - CH_BNDS[c])],
                     f16, name=f"pooled4c{c}")
            for c in range(NCHUNK)
        ]
        hiddenc = [
            big.tile([32, CH_BNDS[c + 1] - CH_BNDS[c]], f16,
                     name=f"hiddenc{c}") for c in range(NCHUNK)
        ]
        outb = big.tile([1, SEG_PER_CORE], f16, name="outb")

        # ---------------------------------------------------------- trees
        def _cost(kind, eng, cols):
            per, fix = {
                ("tt", "dve"): C_DVE_TT, ("cp", "dve"): C_DVE_CP,
                ("tt", "gps"): C_POOL_TT, ("cp", "gps"): C_POOL_CP,
            }[(kind, eng)]
            return per * cols + fix

        # the whole tree of a group runs on ONE engine (cross-engine level
        # chains head-of-line-block the DVE stream)
        tree_eng = {"eng": "dve", "force_dve": False}

        def _emit(kind, out_ap, in0_ap, in1_ap, cols):
            eng = tree_eng["eng"]
            eng_ns[eng] += _cost(kind, eng, cols)
            if eng == "gps":
                if kind == "tt":
                    nc.gpsimd.tensor_tensor(out_ap, in0_ap, in1_ap, add)
                else:
                    nc.gpsimd.tensor_copy(out_ap, in0_ap)
            else:
                if kind == "tt":
                    nc.vector.tensor_tensor(out_ap, in0_ap, in1_ap, add)
                else:
                    nc.vector.tensor_copy(out_ap, in0_ap)

        def tree_op(kind, out_ap, in0_ap, in1_ap, cols, n):
            """APs shaped [p, s, n, a]; split along n when large."""
            if cols <= TREE_CHUNK or n <= 1:
                _emit(kind, out_ap, in0_ap, in1_ap, cols)
                return
            pieces = -(-cols // TREE_CHUNK)
            step = -(-n // pieces)
            for n0 in range(0, n, step):
                n1 = min(n, n0 + step)
                sl = (slice(None), slice(None), slice(n0, n1))
                _emit(kind, out_ap[sl], in0_ap[sl],
                      None if in1_ap is None else in1_ap[sl],
                      cols * (n1 - n0) // n)

        def emit_tree(gi):
            m, j0, n, a0 = groups[gi]
            ri = region_of[gi]
            md, ra0, ra1, _gis = regions[ri]
            t = rtiles[ri]
            off = a0 - ra0
            # pick the engine for this group's whole tree
            total = 2.0 * n * m * (2 if md else 1)
            cd = total * C_DVE_TT[0]
            cg = total * C_POOL_TT[0]
            if (not tree_eng["force_dve"]
                    and eng_ns["gps"] + cg < eng_ns["dve"] + cd):
                tree_eng["eng"] = "gps"
            else:
                tree_eng["eng"] = "dve"
            if md:
                v = t[:].rearrange("p (s ab x) -> p s ab x", s=2, ab=2)
                vA = v[:, :, 0, off : off + n * m].rearrange(
                    "p s (n m) -> p s n m", m=m)
                vB = v[:, :, 1, off : off + n * m].rearrange(
                    "p s (n m) -> p s n m", m=m)
                # first level: A-block += B-block (both sides)
                tree_op("tt", vA, vA, vB, 2 * n * m, n)
                sub = vA
            else:
                sub = t[:].rearrange("p (s x) -> p s x", s=2)[
                    :, :, off : off + n * m
                ].rearrange("p s (n m) -> p s n m", m=m)
            wt_ = 2 * min(folds)
            w = m
            while w > wt_:
                a = (w // 2) if w >= 2 * wt_ + 1 else (w - wt_)
                tree_op("tt", sub[:, :, :, 0:a], sub[:, :, :, 0:a],
                        sub[:, :, :, w - a : w], 2 * n * a, n)
                w -= a
            j = j0
            while j < j0 + n:
                c = chunk_index(j)
                j2 = min(j0 + n, CH_BNDS[c + 1])
                fd = folds[c]
                p4 = pooled4c[c][:].rearrange(
                    "p (s j q) -> p s j q", s=2, q=fd
                )
                pout = p4[:, :, j - CH_BNDS[c] : j2 - CH_BNDS[c], :]
                nn = j2 - j
                sv = sub[:, :, j - j0 : j2 - j0]
                if w == 2 * fd:
                    tree_op("tt", pout, sv[:, :, :, 0:fd],
                            sv[:, :, :, fd : 2 * fd], 2 * nn * fd, nn)
                elif w == fd:
                    tree_op("cp", pout, sv[:, :, :, 0:fd], None,
                            2 * nn * fd, nn)
                elif w < fd:
                    raise AssertionError("w < fold")
                else:
                    a = w - fd
                    tree_op("tt", pout[:, :, :, 0:a], sv[:, :, :, 0:a],
                            sv[:, :, :, w - a : w], 2 * nn * a, nn)
                    tree_op("cp", pout[:, :, :, a:fd],
                            sv[:, :, :, a : w - a], None,
                            2 * nn * (2 * fd - w), nn)
                j = j2

        # ------------------------------------------------------- combine
        ppA = ctx.enter_context(
            tc.tile_pool(name="psA", bufs=2, space="PSUM"))
        ppB = ctx.enter_context(
            tc.tile_pool(name="psB", bufs=2, space="PSUM"))

        combine_ph = {}

        def emit_combine_mm(c):
            fd = folds[c]
            ln = CH_BNDS[c + 1] - CH_BNDS[c]
            p4v = pooled4c[c][:].rearrange(
                "p (s j q) -> p s j q", s=2, q=fd
            )
            ph = ppB.tile([128, 1024], f32, tag="ptB")
            combine_ph[c] = ph
            nmm = 2 * fd
            i = 0
            for s_i in range(2):
                wc = wt[:, 128 + 128 * s_i : 256 + 128 * s_i]
                for qq in range(fd):
                    nc.tensor.matmul(
                        ph[:, 0:ln], wc, p4v[:, s_i, :, qq],
                        start=(i == 0), stop=(i == nmm - 1),
                    )
                    i += 1

        # the last two (256-wide) chunks share one mm2/sigmoid/DMA stage
        # so the final tail has a single output chain
        TAIL_MERGE = (NCHUNK - 2, NCHUNK - 1)

        def emit_combine_post(c):
            ln = CH_BNDS[c + 1] - CH_BNDS[c]
            ph = combine_ph.pop(c)
            nc.scalar.activation(
                hiddenc[c][:], ph[0:32, 0:ln], relu, bias=bt[0:32, 0:1]
            )
            eng_ns["act"] += 0.833 * ln + 187.0
            if c == TAIL_MERGE[0]:
                return  # mm2/sigmoid/DMA deferred into the last chunk
            pl = ppB.tile([128, 1024], f32, tag="ptB")
            if c == TAIL_MERGE[1]:
                c0 = TAIL_MERGE[0]
                l0 = CH_BNDS[c0 + 1] - CH_BNDS[c0]
                nc.tensor.matmul(
                    pl[:, 0:l0], wt[0:32, 384:512], hiddenc[c0][:],
                    start=True, stop=True,
                )
                nc.tensor.matmul(
                    pl[:, l0 : l0 + ln], wt[0:32, 384:512], hiddenc[c][:],
                    start=True, stop=True,
                )
                lo, hi = CH_BNDS[c0], CH_BNDS[c + 1]
                nc.scalar.activation(
                    outb[:, lo:hi], pl[0:1, 0 : hi - lo],
                    sigmoid, bias=bt[0:1, 1:2],
                )
                nc.sync.dma_start(out_dram[:, lo:hi], outb[:, lo:hi])
                return
            nc.tensor.matmul(
                pl[:, 0:ln], wt[0:32, 384:512], hiddenc[c][:],
                start=True, stop=True,
            )
            nc.scalar.activation(
                outb[:, CH_BNDS[c] : CH_BNDS[c + 1]], pl[0:1, 0:ln],
                sigmoid, bias=bt[0:1, 1:2],
            )
            nc.sync.dma_start(
                out_dram[:, CH_BNDS[c] : CH_BNDS[c + 1]],
                outb[:, CH_BNDS[c] : CH_BNDS[c + 1]],
            )

        # group -> chunks it feeds; chunk ready-count
        chunk_need = [0] * NCHUNK
        group_chunks = []
        for (m, j0, n, a0) in groups:
            cs = range(chunk_index(j0), chunk_index(j0 + n - 1) + 1)
            group_chunks.append(list(cs))
            for c in cs:
                chunk_need[c] += 1

        # ------------------------------------------------- transit windows
        def h16_flat(ri, s_i, lo, hi):
            """Contiguous (fused) region h16 slice for alphas [lo,hi)."""
            md, ra0, ra1, _g = regions[ri]
            base = (ra1 - ra0) * s_i
            return rtiles[ri][:, base + lo - ra0 : base + hi - ra0]

        def h16_ab(ri, s_i, ab, lo, hi):
            """Contiguous region h16 slice in the A(0)/B(1) block of a
            split region."""
            md, ra0, ra1, _g = regions[ri]
            ra = ra1 - ra0
            base = (2 * s_i + ab) * ra
            return rtiles[ri][:, base + lo - ra0 : base + hi - ra0]

        def region_pieces(a_lo, a_hi):
            out = []
            pos = a_lo
            for ri, (md, ra0, ra1, _g) in enumerate(regions):
                if ra1 <= pos or ra0 >= a_hi:
                    continue
                lo, hi = max(pos, ra0), min(a_hi, ra1)
                if lo > pos:
                    out.append((None, pos, lo))
                out.append((ri, lo, hi))
                pos = hi
            if pos < a_hi:
                out.append((None, pos, a_hi))
            return out

        def sub512(lo, hi, base):
            """Split [lo,hi) at 512 boundaries rel. to base -> list of
            (l2, h2, is_full_supertile)."""
            cuts = [lo]
            b = base + (-(-(lo - base) // 512)) * 512
            while b < hi:
                if b > cuts[-1]:
                    cuts.append(b)
                b += 512
            if hi > cuts[-1]:
                cuts.append(hi)
            return [
                (cuts[i], cuts[i + 1],
                 (cuts[i] - base) % 512 == 0
                 and (cuts[i + 1] - cuts[i]) == 512)
                for i in range(len(cuts) - 1)
            ]

        def merged_runs(lo, hi, base):
            """sub512 runs with consecutive full supertiles merged ->
            [(l2, h2, st, stn)]."""
            runs = sub512(lo, hi, base)
            out = []
            i = 0
            while i < len(runs):
                l2, h2, fl = runs[i]
                if fl:
                    j2 = i
                    while j2 + 1 < len(runs) and runs[j2 + 1][2]:
                        j2 += 1
                    out.append((l2, runs[j2][1], (l2 - base) // 512,
                                j2 - i + 1))
                    i = j2 + 1
                else:
                    out.append((l2, h2, (l2 - base) // 512, 0))
                    i += 1
            return out

        # warm the PE p-state early with a dependency-free matmul (memset
        # inputs, no DMA wait) so the first real matmuls run full speed
        wz = consts.tile([32, 128], f16)
        nc.vector.memset(wz[:], 0)
        warm_pt = ppA.tile([128, 1024], f32, tag="ptA")
        nc.tensor.matmul(warm_pt[0:64, 0:64], wz[0:20, 0:64],
                         wz[0:20, 64:128], start=True, stop=True)

        n_stp = n_st + 2
        n_win = (n_stp + 1) // 2
        A_end = A_tot
        ring_hist = {0: [], 1: []}  # per side, ring tiles by window index
        combine_q = []  # (emit_at_window, chunk)
        for wi in range(n_win):
            sts = [p for p in (2 * wi, 2 * wi + 1) if p < n_stp]
            nsts = len(sts)
            base_B = 512 * 2 * wi
            hi_B = min(base_B + 512 * nsts, A_end)
            base_A = 512 * (2 * wi - 2)
            hi_A = min(base_A + 512 * nsts, A_end)
            piecesB = region_pieces(base_B, hi_B) if hi_B > base_B else []
            piecesA = (region_pieces(base_A, hi_A)
                       if hi_A > max(base_A, 0) else [])
            ptvAs, ptvBs = {}, {}
            for s_i, s in enumerate(("L", "R")):
                ptA = ppA.tile([128, 1024], f32, tag="ptA")
                ptB = ppB.tile([128, 1024], f32, tag="ptB")
                for i, p in enumerate(sts):
                    b = band_of(p)
                    colA = (p - st_edges[b]) * 1024
                    wap = wt[32 * b : 32 * b + 20, 0:128]
                    if 2 <= p and (p - 2) * 512 < A2:
                        nc.tensor.matmul(
                            ptA[:, 512 * i : 512 * i + 512], wap,
                            xt[s][32 * b : 32 * b + 20, colA : colA + 512],
                            start=True, stop=True,
                            tile_position=(32 * b, 0),
                        )
                for i, p in enumerate(sts):
                    b = band_of(p)
                    colA = (p - st_edges[b]) * 1024
                    wap = wt[32 * b : 32 * b + 20, 0:128]
                    if p < n_st:
                        nc.tensor.matmul(
                            ptB[:, 512 * i : 512 * i + 512], wap,
                            xt[s][32 * b : 32 * b + 20,
                                  colA + 512 : colA + 1024],
                            start=True, stop=True,
                            tile_position=(32 * b, 0),
                        )
                ptvAs[s_i] = ptA[:].rearrange("p (st c) -> p st c", st=2)
                ptvBs[s_i] = ptB[:].rearrange("p (st c) -> p st c", st=2)

            for s_i, s in enumerate(("L", "R")):
                ptvA = ptvAs[s_i]
                ptvB = ptvBs[s_i]
                # ---- A banks first: STT with ring from last window ----
                for (ri, lo, hi) in piecesA:
                    if ri is None:
                        continue
                    prg = ring_hist[s_i][wi - 1]
                    for (l2, h2, st, stn) in merged_runs(lo, hi, base_A):
                        if regions[ri][0]:
                            dst = h16_ab(ri, s_i, 0, l2, h2)
                            if stn == 0:
                                c0 = l2 - base_A - 512 * st
                                src = ptvA[:, st, c0 : c0 + h2 - l2]
                            else:
                                src = ptvA[:, st : st + stn, :]
                                dst = dst.rearrange("p (st c) -> p st c",
                                                    st=stn)
                            nc.scalar.activation(dst, src, relu)
                            eng_ns["act"] += 0.833 * (h2 - l2) + 187.0
                        else:
                            dst = h16_flat(ri, s_i, l2, h2)
                            if stn == 0:
                                c0 = l2 - base_A - 512 * st
                                src = ptvA[:, st, c0 : c0 + h2 - l2]
                                rr = prg[:, l2 - base_A : h2 - base_A]
                            else:
                                src = ptvA[:, st : st + stn, :]
                                dst = dst.rearrange("p (st c) -> p st c",
                                                    st=stn)
                                rr = prg[:, l2 - base_A : h2 - base_A]\
                                    .rearrange("p (st c) -> p st c", st=stn)
                            nc.vector.scalar_tensor_tensor(
                                dst, src, 0.0, rr, mx, add)
                            eng_ns["dve"] += 1.042 * (h2 - l2) + 125.0

            for s_i, s in enumerate(("L", "R")):
                ptvB = ptvBs[s_i]
                # ---- B banks: partners of the alphas 1 window ahead ----
                rg = rgv = None
                if any(ri is not None and not regions[ri][0]
                       for (ri, _, _) in piecesB):
                    rg = ring_pool.tile([128, 1024], f16, tag="ring")
                    rgv = rg[:].rearrange("p (st c) -> p st c", st=2)
                runsB = []
                for (ri, lo, hi) in piecesB:
                    if ri is None:
                        continue
                    md = regions[ri][0]
                    if runsB and not md and not runsB[-1][0] \
                            and runsB[-1][2] == lo:
                        runsB[-1] = (False, runsB[-1][1], hi, None)
                    else:
                        runsB.append((md, lo, hi, ri))
                for (md, lo, hi, ri) in runsB:
                    for (l2, h2, st, stn) in merged_runs(lo, hi, base_B):
                        if stn == 0:
                            c0 = l2 - base_B - 512 * st
                            src = ptvB[:, st, c0 : c0 + h2 - l2]
                            dst = (h16_ab(ri, s_i, 1, l2, h2) if md
                                   else rg[:, l2 - base_B : h2 - base_B])
                        else:
                            src = ptvB[:, st : st + stn, :]
                            dst = (h16_ab(ri, s_i, 1, l2, h2).rearrange(
                                "p (st c) -> p st c", st=stn) if md
                                else rgv[:, st : st + stn, :])
                        nc.scalar.activation(dst, src, relu)
                        eng_ns["act"] += 0.833 * (h2 - l2) + 187.0
                ring_hist[s_i].append(rg)
                # after the R side, emit trees for regions ending here
                if s_i == 1:
                    tree_eng["force_dve"] = wi >= n_win - 2
                    for (ri, lo, hi) in piecesA:
                        if ri is None:
                            continue
                        if hi == regions[ri][2]:
                            for gi in regions[ri][3]:
                                emit_tree(gi)
                                for c in group_chunks[gi]:
                                    chunk_need[c] -= 1
                                    if chunk_need[c] == 0:
                                        combine_q.append([wi + 5, 0, c])
            # staged combine emission: matmuls 3 windows after readiness,
            # relu/mm2/sigmoid one window later (avoids ACT head-of-line)
            for ent in list(combine_q):
                due, stage, c = ent
                if due <= wi:
                    if stage == 0:
                        emit_combine_mm(c)
                        ent[0] = wi
                        ent[1] = 1
                    else:
                        emit_combine_post(c)
                        combine_q.remove(ent)
        for (_due, stage, c) in sorted(combine_q, key=lambda e: e[1],
                                       reverse=True):
            if stage == 0:
                emit_combine_mm(c)
                emit_combine_post(c)
            else:
                emit_combine_post(c)

    nc.compile()
    return nc


# ------------------------------------------------------------------- driver

def kernel(**inputs):
    meta = host_prep(
        inputs["left_feats"], inputs["right_feats"],
        inputs["left_seg"], inputs["right_seg"],
    )
    wab = make_weight_arrays(
        inputs["W1"], inputs["b1"], inputs["Wc1"], inputs["bc1"],
        inputs["Wc2"], inputs["bc2"],
    )
    nc = build_nc(meta["A2"], meta["n_st"], meta["aux"], meta["groups"])
    in_maps = []
    for d in range(N_CORES):
        c = meta["cores"][d]
        in_maps.append(dict(xhL=c["xhL"], xhR=c["xhR"], wts=wab["wts"],
                            bias=wab["bias"]))
    res = run_bass_kernel_spmd(nc, in_maps, core_ids=list(range(N_CORES)))
    global _last_results
    _last_results = res
    out = np.zeros(BATCH, dtype=np.float32)
    for d in range(N_CORES):
        ids = meta["cores"][d]["ids"]
        dev = np.asarray(res.results[d]["out"]).reshape(-1).astype(np.float32)
        out[ids] = dev
    return out
